# revision 46
# baseline (speedup 1.0000x reference)
"""Trainium2 Bass kernel for batched multi-head attention (B=8, T=2048, C=1024, H=16).

Sharding: data-parallel over batch - one batch element per NeuronCore (8 cores).

Per-core algorithm (v3):
  Projections (Q/K/V) run as fp8e4 DoubleRow 3-chain matmuls:
      y = x_hi@W_hi + x_hi@W_lo + x_lo@W_hi      (x_lo@W_lo dropped)
  where x_hi = fp8(x), x_lo = fp8(x - x_hi), same for W (host-prepared).
  Each chain contracts C=1024 as 4 DoubleRow instructions (2x128 each) at
  0.5 cycles/row -> 12 instructions per output tile vs 8 full-rate bf16 ones
  (25% less PE time) and ~2x better accuracy than bf16 inputs.
  W_q/W_k are host-scaled by 16 (fp8 normal range), W_v by 32; the softmax
  scale becomes 0.125/256 and the V de-scale rides the existing mask
  multiply (mv = valid/32).

  S^T[ki, qi] per (head, kt): plain bf16 matmul, K=64 (d), M=128 keys,
  N=512 queries.  q/k stay bf16: fp8 q/k noise is coherent (acts like a
  perturbed query) and lands ~5% on the output - measured, not acceptable.

  exp: split between ACT (activation Exp) and Pool (tensor_tensor pow base e;
  gpsimd cannot read PSUM so a DVE tensor_scalar stages scale*S^T in SBUF).
  AV: stationary P^T [128 ki, 128 qi] (SBUF bf16), moving v_aug [128 ki, 65]
  (col 64 = valid/32) -> PSUM [128 qi, 4 qs, 65] accumulated over kt; col 64
  accumulates l/32 which cancels against v/32 in the final divide.  Output is
  produced directly in [qi, d] orientation: no transposes; finish = strided
  reciprocal of l + broadcast multiply + DMA.

  K/V positions are host-compacted to the mask==1 subset (padded to TK).
"""

import sys

sys.path.insert(0, "/opt/trn_rl_repo")

from contextlib import ExitStack

import numpy as np
import ml_dtypes

import concourse.bass as bass  # noqa: F401
import concourse.tile as tile
from concourse import bacc, mybir
from concourse.bass_utils import run_bass_kernel_spmd

B, T, C, H, D = 8, 2048, 1024, 16, 64
NCORES = 8
BF16 = mybir.dt.bfloat16
F32 = mybir.dt.float32
FP8 = mybir.dt.float8e4
DR = mybir.MatmulPerfMode.DoubleRow

COMPACT = True
TK_COMPACT = 1152

QK_SCALE = 16.0
V_SCALE = 32.0
SOFT_SCALE = 0.125 / (QK_SCALE * QK_SCALE)
# P is stored fp8e4 (max 240): shift the exponent down so exp() stays in
# range; the e^-c0 factor cancels between numerator and l
EXP_C0 = 2.0

WARM_UNITS = 2  # units whose S/exp interleave with the V projection
PT_BUFS = 13
POOL_SLOTS = (2, 5, 8)  # ctr % 10 in these -> exp on Pool (pow)
GRP = 2  # kt blocks per S psum group
S_BUFS = (8 - 2) // GRP  # PSUM: GRP*S_BUFS banks for S + 2 for AV

_nc_cache = {}


def build_nc(TK):
    KT = TK // 128
    nc = bacc.Bacc(None)

    xth_d = nc.dram_tensor("xth", [8, 128, T], FP8, kind="ExternalInput")
    xtl_d = nc.dram_tensor("xtl", [8, 128, T], FP8, kind="ExternalInput")
    xch_d = nc.dram_tensor("xch", [8, 128, TK], FP8, kind="ExternalInput")
    xcl_d = nc.dram_tensor("xcl", [8, 128, TK], FP8, kind="ExternalInput")
    wqkh_d = nc.dram_tensor("wqkh", [8, 128, 2048], FP8, kind="ExternalInput")
    wqkl_d = nc.dram_tensor("wqkl", [8, 128, 2048], FP8, kind="ExternalInput")
    wvh_d = nc.dram_tensor("wvh", [8, 128, 1024], FP8, kind="ExternalInput")
    wvl_d = nc.dram_tensor("wvl", [8, 128, 1024], FP8, kind="ExternalInput")
    bqk_d = nc.dram_tensor("bqk", [128, 16], F32, kind="ExternalInput")
    mv_d = nc.dram_tensor("mv", [128, KT], F32, kind="ExternalInput")
    out_d = nc.dram_tensor("out", [H * T, D], F32, kind="ExternalOutput")

    with tile.TileContext(nc) as tc, ExitStack() as ctx:
        const = ctx.enter_context(tc.tile_pool(name="const", bufs=1))

        # --- input DMAs: one per tensor (HWDGE issue costs ~625ns per DMA),
        # in consumption-priority order: V inputs, then hg0 q/k, then the rest
        xch = const.tile([128, 8, TK], FP8)
        xcl = const.tile([128, 8, TK], FP8)
        wvh = const.tile([128, 8, 1024], FP8)
        wvl = const.tile([128, 8, 1024], FP8)
        nc.sync.dma_start(xch[:, 0:4, :], xch_d[0:4].rearrange("c p t -> p c t"))
        nc.gpsimd.dma_start(wvh[:, 0:4, :], wvh_d[0:4].rearrange("c p t -> p c t"))
        nc.sync.dma_start(xch[:, 4:8, :], xch_d[4:8].rearrange("c p t -> p c t"))
        nc.gpsimd.dma_start(wvh[:, 4:8, :], wvh_d[4:8].rearrange("c p t -> p c t"))
        nc.sync.dma_start(xcl[:, 0:4, :], xcl_d[0:4].rearrange("c p t -> p c t"))
        nc.sync.dma_start(xcl[:, 4:8, :], xcl_d[4:8].rearrange("c p t -> p c t"))
        nc.gpsimd.dma_start(wvl[:], wvl_d[:].rearrange("c p t -> p c t"))
        mv = const.tile([128, KT], F32)
        nc.sync.dma_start(mv[:], mv_d[:])
        bqk = const.tile([128, 16], F32)
        nc.sync.dma_start(bqk[:], bqk_d[:])
        xth = const.tile([128, 8, T], FP8)
        xtl = const.tile([128, 8, T], FP8)
        nc.sync.dma_start(xth[:], xth_d[:].rearrange("c p t -> p c t"))
        nc.sync.dma_start(xtl[:], xtl_d[:].rearrange("c p t -> p c t"))
        wqkh = const.tile([128, 8, 2048], FP8)
        wqkl = const.tile([128, 8, 2048], FP8)
        # head-group 0 slices (q cols 0:256, k cols 1024:1280) first
        nc.sync.dma_start(wqkh[:, :, 0:256], wqkh_d[:, :, 0:256].rearrange("c p t -> p c t"))
        nc.sync.dma_start(wqkl[:, :, 0:256], wqkl_d[:, :, 0:256].rearrange("c p t -> p c t"))
        nc.sync.dma_start(wqkh[:, :, 1024:1280], wqkh_d[:, :, 1024:1280].rearrange("c p t -> p c t"))
        nc.sync.dma_start(wqkl[:, :, 1024:1280], wqkl_d[:, :, 1024:1280].rearrange("c p t -> p c t"))
        nc.sync.dma_start(wqkh[:, :, 256:1024], wqkh_d[:, :, 256:1024].rearrange("c p t -> p c t"))
        nc.sync.dma_start(wqkh[:, :, 1280:2048], wqkh_d[:, :, 1280:2048].rearrange("c p t -> p c t"))
        nc.sync.dma_start(wqkl[:, :, 256:1024], wqkl_d[:, :, 256:1024].rearrange("c p t -> p c t"))
        nc.sync.dma_start(wqkl[:, :, 1280:2048], wqkl_d[:, :, 1280:2048].rearrange("c p t -> p c t"))

        cpow = const.tile([128, GRP, 512], BF16)
        nc.vector.memset(cpow[:], float(np.e))
        # exact base compensation: pow base is bf16(e), so scale by 1/ln(base)
        c_eff = float(np.asarray(np.e, dtype=ml_dtypes.bfloat16))
        pow_scale = SOFT_SCALE / float(np.log(c_eff))
        sscale = const.tile([128, 1], F32)
        nc.vector.memset(sscale[:], pow_scale)
        pow_scale_f = float(pow_scale)
        stage_ctr = [0]
        nbias = const.tile([128, 1], F32)
        nc.vector.memset(nbias[:], -EXP_C0)

        qT = const.tile([128, 8, T], BF16)
        kT = const.tile([128, 8, TK], BF16)
        vsb = const.tile([128, KT, 16, 65], BF16)
        # l-column seed: V_SCALE cancels the valid/V_SCALE in mv so the l
        # accumulator sees plain `valid` while data columns see v*valid
        nc.vector.memset(vsb[:, :, :, 64:65], V_SCALE)

        psum = ctx.enter_context(tc.tile_pool(name="psum", bufs=1, space="PSUM"))
        sb = ctx.enter_context(tc.tile_pool(name="sb", bufs=1))

        CHAINS = [(xth, wqkh), (xth, wqkl), (xtl, wqkh)]
        VCHAINS = [(xch, wvh), (xcl, wvh), (xch, wvl)]
        KCHAINS = [(xch, wqkh), (xch, wqkl), (xcl, wqkh)]

        # ---------- emission helpers ----------
        def qk_proj_pieces(hg):
            """One thunk per projection output tile for chunks 2hg, 2hg+1."""
            pieces = []
            for ci in (2 * hg, 2 * hg + 1):
                for tb in range(T // 512):
                    def qp(ci=ci, tb=tb):
                        ps = psum.tile([128, 1, 512], F32, tag="av", bufs=2, name="ps")
                        n = 0
                        for xa, wa in CHAINS:
                            for u in range(4):
                                nc.tensor.matmul(
                                    ps[:, 0, :],
                                    wa[:, 2 * u : 2 * u + 2, ci * 128 : (ci + 1) * 128],
                                    xa[:, 2 * u : 2 * u + 2, tb * 512 : (tb + 1) * 512],
                                    start=(n == 0),
                                    stop=(n == 11),
                                    perf_mode=DR,
                                )
                                n += 1
                        nc.vector.tensor_scalar_add(
                            out=qT[:, ci, tb * 512 : (tb + 1) * 512],
                            in0=ps[:, 0, :],
                            scalar1=bqk[:, ci : ci + 1],
                        )
                    pieces.append(qp)
                for t0 in range(0, TK, 512):
                    def kp(ci=ci, t0=t0):
                        w = min(512, TK - t0)
                        ps = psum.tile([128, 1, 512], F32, tag="av", bufs=2, name="ps")
                        n = 0
                        for xa, wa in KCHAINS:
                            for u in range(4):
                                nc.tensor.matmul(
                                    ps[:, 0, :w],
                                    wa[:, 2 * u : 2 * u + 2, 1024 + ci * 128 : 1024 + (ci + 1) * 128],
                                    xa[:, 2 * u : 2 * u + 2, t0 : t0 + w],
                                    start=(n == 0),
                                    stop=(n == 11),
                                    perf_mode=DR,
                                )
                                n += 1
                        nc.vector.tensor_scalar_add(
                            out=kT[:, ci, t0 : t0 + w],
                            in0=ps[:, 0, :w],
                            scalar1=bqk[:, 8 + ci : 9 + ci],
                        )
                    pieces.append(kp)
            return pieces

        def v_step(ti, nn):
            ps = psum.tile([128, GRP, 512], F32, tag="s", bufs=S_BUFS)
            n = 0
            for xa, wa in VCHAINS:
                for u in range(4):
                    nc.tensor.matmul(
                        ps[:, 0, :],
                        xa[:, 2 * u : 2 * u + 2, ti * 128 : (ti + 1) * 128],
                        wa[:, 2 * u : 2 * u + 2, nn * 512 : (nn + 1) * 512],
                        start=(n == 0),
                        stop=(n == 11),
                        perf_mode=DR,
                    )
                    n += 1
            nc.vector.tensor_scalar_mul(
                out=vsb[:, ti, nn * 8 : (nn + 1) * 8, 0:64],
                in0=ps[:, 0, :].rearrange("p (h d) -> p h d", h=8),
                scalar1=mv[:, ti : ti + 1],
            )

        exp_ctr = [0]

        def s_groups(h, qi, warm):
            """S matmuls + exp for one (head, qi512) unit; returns pt groups."""
            ci, r0 = h // 2, (h % 2) * 64
            return [
                s_one_group(ci, r0, qi, g0, warm) for g0 in range(0, KT, GRP)
            ]

        def s_one_group(ci, r0, qi, g0, warm, use_pool=None, weave=None):
            if True:
                kts = list(range(g0, min(g0 + GRP, KT)))
                gl = len(kts)
                ps = psum.tile([128, GRP, 512], F32, tag="s", bufs=S_BUFS)
                for idx, kt in enumerate(kts):
                    if weave is not None and idx < len(weave):
                        weave[idx]()
                    nc.tensor.matmul(
                        ps[:, idx, :],
                        kT[r0 : r0 + 64, ci, kt * 128 : (kt + 1) * 128],
                        qT[r0 : r0 + 64, ci, qi * 512 : (qi + 1) * 512],
                        start=True,
                        stop=True,
                    )
                if weave is not None:
                    for wv in weave[len(kts):]:
                        wv()
                pt = sb.tile([128, GRP, 512], BF16, tag="pt", bufs=PT_BUFS)
                if use_pool is None:
                    use_pool = (not warm) and (exp_ctr[0] % 10 in POOL_SLOTS)
                exp_ctr[0] += 1
                if use_pool:
                    sx = sb.tile([128, GRP, 512], BF16, tag="sx", bufs=2)
                    nc.vector.tensor_scalar_mul(
                        out=sx[:, :gl, :], in0=ps[:, :gl, :], scalar1=sscale[:]
                    )
                    nc.gpsimd.tensor_tensor(
                        out=pt[:, :gl, :],
                        in0=cpow[:, :gl, :],
                        in1=sx[:, :gl, :],
                        op=mybir.AluOpType.pow,
                    )
                else:
                    nc.scalar.activation(
                        out=pt[:, :gl, :],
                        in_=ps[:, :gl, :],
                        func=mybir.ActivationFunctionType.Exp,
                        scale=SOFT_SCALE,
                    )
                return (pt, kts, use_pool)

        def av_kt(h, av, pt, idx, kt, pos, total):
            for qs in range(4):
                pos += 1
                nc.tensor.matmul(
                    av[:, qs, :],
                    pt[:, idx, qs * 128 : (qs + 1) * 128],
                    vsb[:, kt, h, :],
                    start=(pos == 1),
                    stop=(pos == total),
                    skip_group_check=True,
                )
            return pos

        def av_groups(h, av, groups, sel, pos0, total):
            # AV matmuls for the selected group indices; one accumulation
            # group per av bank (start_tensor_calc zeroes the whole bank);
            # group order is free (it is a sum), start/stop by position
            pos = pos0
            for gi in sel:
                pt, kts, _ = groups[gi]
                for idx, kt in enumerate(kts):
                    for qs in range(4):
                        pos += 1
                        nc.tensor.matmul(
                            av[:, qs, :],
                            pt[:, idx, qs * 128 : (qs + 1) * 128],
                            vsb[:, kt, h, :],
                            start=(pos == 1),
                            stop=(pos == total),
                            skip_group_check=True,
                        )
            return pos

        def finish(h, qi, av):
            linv = sb.tile([128, 4], F32, tag="li", bufs=4)
            nc.vector.reciprocal(linv[:], av[:, :, 64])
            of = sb.tile([128, 4, 64], F32, tag="of", bufs=4)
            nc.vector.tensor_tensor(
                out=of[:],
                in0=av[:, :, 0:64],
                in1=linv[:, :, None].broadcast_to([128, 4, 64]),
                op=mybir.AluOpType.mult,
            )
            dst = out_d[h * T + qi * 512 : h * T + (qi + 1) * 512, :].rearrange(
                "(q p) d -> p q d", p=128
            )
            nc.sync.dma_start(dst, of[:])

        def av_finish(h, qi, groups):
            av = psum.tile([128, 4, 65], F32, tag="av", bufs=2)
            total = 4 * sum(len(g[1]) for g in groups)
            order = [gi for gi, g in enumerate(groups) if not g[2]] + [
                gi for gi, g in enumerate(groups) if g[2]
            ]
            av_groups(h, av, groups, order, 0, total)
            finish(h, qi, av)

        # ---------- main emission ----------
        units = [(4 * hg + hh, qi) for hg in range(4) for hh in range(4) for qi in range(4)]
        pts_map = {}

        # V projection first (its inputs arrive first); head-group 0's q/k
        # projection pieces join once their DMAs have landed, then warm
        # S/exp units so ACT spins up before the V projection finishes
        hg0 = qk_proj_pieces(0)
        hg0_done = 0
        wi = 0
        for ti in range(KT):
            for nn in range(2):
                v_step(ti, nn)
            nc.vector.tensor_scalar_mul(
                out=vsb[:, ti, :, 64:65],
                in0=vsb[:, ti, :, 64:65],
                scalar1=mv[:, ti : ti + 1],
            )
            if ti >= 4:
                want = min((ti - 3) * len(hg0) * 2 // (KT - 4), len(hg0))
                while hg0_done < want:
                    hg0[hg0_done]()
                    hg0_done += 1
            if hg0_done >= 7:
                while wi < min(WARM_UNITS, 2 * (hg0_done - 6)):
                    u = units[wi]
                    pts_map[u] = s_groups(u[0], u[1], warm=True)
                    wi += 1
        while hg0_done < len(hg0):
            hg0[hg0_done]()
            hg0_done += 1
        while wi < WARM_UNITS:
            u = units[wi]
            pts_map[u] = s_groups(u[0], u[1], warm=True)
            wi += 1

        # steady state: S(u) ... AV(u-1), with next head-group's projections
        # spread between units
        pieces_by_hg = {hg: qk_proj_pieces(hg) for hg in (1, 2, 3)}
        emitted = {hg: 0 for hg in (1, 2, 3)}

        def pump(hgn, target):
            if hgn not in pieces_by_hg:
                return
            pcs = pieces_by_hg[hgn]
            t = min(target, len(pcs))
            while emitted[hgn] < t:
                pcs[emitted[hgn]]()
                emitted[hgn] += 1

        prev = None
        for ui, u in enumerate(units):
            hg = ui // 16
            if hg >= 1:
                pump(hg, 99)  # this head-group's projections must be complete
            if u in pts_map:
                if prev is not None:
                    av_finish(prev[0], prev[1], pts_map.pop(prev))
            else:
                # emit u's S/exp groups with prev's AV groups woven between
                pgroups = pts_map.pop(prev) if prev is not None else None
                if pgroups is not None:
                    pav = psum.tile([128, 4, 65], F32, tag="av", bufs=2, name="pav")
                else:
                    pav = None
                ci, r0 = u[0] // 2, (u[0] % 2) * 64
                groups = []
                if pgroups is not None:
                    # ACT-produced groups first, Pool-produced last so the
                    # slower pow path gets maximal lead time
                    order = [gi for gi, g in enumerate(pgroups) if not g[2]] + [
                        gi for gi, g in enumerate(pgroups) if g[2]
                    ]
                    ptotal = 4 * sum(len(g[1]) for g in pgroups)
                    ppos = 0
                for gi, g0 in enumerate(range(0, KT, GRP)):
                    # AV of prev's oldest-ready group first, then this S group
                    if pgroups is not None and gi < len(order):
                        ppos = av_groups(
                            prev[0], pav, pgroups, [order[gi]], ppos, ptotal
                        )
                    up = gi in (0, 2)
                    groups.append(
                        s_one_group(ci, r0, u[1], g0, False, use_pool=up)
                    )
                pts_map[u] = groups
                if pgroups is not None:
                    finish(prev[0], prev[1], pav)
            if hg + 1 <= 3:
                pump(hg + 1, ((ui % 16) + 1) * 14 // 16 + 1)
            prev = u
        av_finish(prev[0], prev[1], pts_map.pop(prev))

    nc.compile()
    return nc


def _hi_lo(a):
    hi = a.astype(ml_dtypes.float8_e4m3)
    lo = (a - hi.astype(np.float32)).astype(ml_dtypes.float8_e4m3)
    return hi, lo


def make_in_maps(x, mask, W_qkv, b_qkv, TK):
    KT = TK // 128
    wqkh, wqkl = _hi_lo(np.ascontiguousarray(W_qkv[:, :2048]) * QK_SCALE)
    wvh, wvl = _hi_lo(np.ascontiguousarray(W_qkv[:, 2048:]) * V_SCALE)
    wqkh = wqkh.reshape(8, 128, 2048)
    wqkl = wqkl.reshape(8, 128, 2048)
    wvh = wvh.reshape(8, 128, 1024)
    wvl = wvl.reshape(8, 128, 1024)
    bqk = (b_qkv[:2048] * QK_SCALE).astype(np.float32).reshape(16, 128).T.copy()
    mask2 = np.asarray(mask).reshape(B, T)

    maps = []
    for b in range(B):
        xTb = np.ascontiguousarray(np.asarray(x[b]).T)  # (C, T) f32
        maskb = mask2[b]
        if TK == T:
            xc = xTb
            valid = maskb.astype(np.float32)
        else:
            sel = np.nonzero(maskb)[0]
            assert len(sel) <= TK, f"compaction overflow: {len(sel)} > {TK}"
            xc = np.zeros((C, TK), np.float32)
            xc[:, : len(sel)] = xTb[:, sel]
            valid = np.zeros(TK, np.float32)
            valid[: len(sel)] = 1.0
        xth, xtl = _hi_lo(xTb)
        xch, xcl = _hi_lo(xc)
        maps.append(
            {
                "xth": xth.reshape(8, 128, T),
                "xtl": xtl.reshape(8, 128, T),
                "xch": xch.reshape(8, 128, TK),
                "xcl": xcl.reshape(8, 128, TK),
                "wqkh": wqkh,
                "wqkl": wqkl,
                "wvh": wvh,
                "wvl": wvl,
                "bqk": bqk,
                "mv": (valid / V_SCALE).reshape(KT, 128).T.copy(),
            }
        )
    return maps


def kernel(x, mask, W_qkv, b_qkv):
    mask2 = np.asarray(mask).reshape(B, T)
    TK = T
    if COMPACT:
        need = int(mask2.sum(axis=1).max())
        if need <= TK_COMPACT:
            TK = TK_COMPACT
    if TK not in _nc_cache:
        _nc_cache[TK] = build_nc(TK)
    nc = _nc_cache[TK]
    in_maps = make_in_maps(np.asarray(x), mask, np.asarray(W_qkv), np.asarray(b_qkv), TK)
    res = run_bass_kernel_spmd(nc, in_maps, core_ids=list(range(NCORES)))
    out = np.stack([res.results[c]["out"] for c in range(NCORES)])  # (B, H*T, D)
    # v-bias passes through softmax exactly (weights sum to 1): add on host
    out = out.reshape(B, H, T, D) + np.asarray(b_qkv)[2048:].reshape(1, H, 1, D)
    return out.astype(np.float32).reshape(B, T, C)


if __name__ == "__main__":
    rng = np.random.default_rng(0)
    x = rng.standard_normal((B, T, C), dtype=np.float32)
    mask = (rng.integers(0, 2, (B, 1, 1, T))).astype(np.int32)
    W = (rng.standard_normal((C, 3 * C), dtype=np.float32) * C**-0.5).astype(np.float32)
    bq = (rng.standard_normal(3 * C, dtype=np.float32) * 0.02).astype(np.float32)
    out = kernel(x, mask, W, bq)
    print("out", out.shape, out.dtype)

